# revision 26
# baseline (speedup 1.0000x reference)
"""Trainium2 Bass kernel for nn_CompositeLoss_91053306675239.

Composite loss = 0.1 * LM cross-entropy( [4,1024,32000] logits ) +
                 1.0 * sum_b detection_loss(image b)   (greedy IoU matching)

Sharding: data-parallel. The LM CE is sharded over the 4096 (B*S) rows:
each of the 8 cores streams 512 rows x 32000 vocab (bf16, 32 MB) from
HBM through the Scalar engine (exp+accum). The per-image detection loss
(tiny inputs, sequential greedy matching) runs on the Vector engine on
every core (cores b%4 own image b; the other copies are ignored) and
OVERLAPS the CE stream: the det instructions are issued first so the
long serial DVE chain runs while DMA+ACT stream the vocab.

The greedy loop runs a host-computed iteration bound: the global max of
the (monotonically shrinking) IoU matrix is non-increasing, so once it
drops below THRESH all later iterations contribute exactly zero to the
loss. The host simulates the greedy matching in fp32 to find that step
and pads by 2 (threshold gaps in the data are ~1e-3 >> fp32 noise).

Host only shards inputs, precomputes gather indices / one-hot layouts
from the integer label inputs, and sums the per-core scalar partials.
"""

import numpy as np

# ---- problem constants (hardcoded per contest contract) ----
B, S, V = 4, 1024, 32000
NV, C, T = 256, 80, 32
NCORES = 8
ROWS = (B * S) // NCORES        # 512 CE rows per core
NBLK = ROWS // 128              # 4 partition-blocks
# graduated chunk plan: small chunks first so the Scalar engine starts
# exp-ing ~4us in instead of waiting for a full 4MB transfer
CE_PLAN = [[4000, 4000, 8000, 8000, 8000]] + [[16000, 16000]] * 3
NCHUNKS = sum(len(p) for p in CE_PLAN)

CLS_W = 0.2
COORD_W = 0.8
IOU_W = 0.7
L1_W = 0.3
LM_W = 0.1
DET_W = 1.0
THRESH = 0.5
EPS = 1e-7
PEN = 0.5 * COORD_W * L1_W + 0.5 * CLS_W   # 0.22
GIOU_C = COORD_W * IOU_W                   # 0.56 constant folded out of L
DEF_NITER = T


def build_nc(niter=DEF_NITER):
    import concourse.bass as bass
    import concourse.bacc as bacc
    import concourse.mybir as mybir
    from concourse.tile import TileContext

    f32 = mybir.dt.float32
    bf16 = mybir.dt.bfloat16
    i32 = mybir.dt.int32
    AF = mybir.ActivationFunctionType
    OP = mybir.AluOpType
    AX = mybir.AxisListType

    # Leave exp/ln mapped only to the combined natural_log_exp set so the
    # table-load pass emits one ACT_TABLE_LOAD instead of one per switch.
    if not getattr(bacc, "_act_tbl_patched", False):
        import concourse.hw_specs as hw_specs
        _orig_tables = hw_specs.get_activation_tables
        _exp = mybir.ActivationFunctionType.from_pwp("exp")
        _ln = mybir.ActivationFunctionType.from_pwp("ln")

        def _merged_tables(arch):
            t = {k: set(v) for k, v in _orig_tables(arch).items()}
            for name, fns in t.items():
                if name != "natural_log_exp_and_others":
                    fns.discard(_exp)
                    fns.discard(_ln)
            return t

        bacc.get_activation_tables = _merged_tables
        bacc._act_tbl_patched = True

    nc = bacc.Bacc()

    # ---- dram I/O ----
    lm = nc.dram_tensor("lm", [ROWS * V], bf16, kind="ExternalInput")
    labidx = nc.dram_tensor("labidx", [128, NBLK], i32, kind="ExternalInput")
    validm = nc.dram_tensor("validm", [128, NBLK], f32, kind="ExternalInput")
    pbf = nc.dram_tensor("pbf", [1, 4 * NV], f32, kind="ExternalInput")
    tbd = nc.dram_tensor("tb", [T, 4], f32, kind="ExternalInput")
    c1hT = nc.dram_tensor("c1hT", [C, T], f32, kind="ExternalInput")  # *CLS_W
    clT = nc.dram_tensor("clT", [C, NV], f32, kind="ExternalInput")
    cld = nc.dram_tensor("cl", [NV, C], f32, kind="ExternalInput")
    iotad = nc.dram_tensor("iota", [T, NV], f32, kind="ExternalInput")
    tbbd = nc.dram_tensor("tbb", [T, 4 * NV], f32, kind="ExternalInput")
    id128d = nc.dram_tensor("id128", [128, 128], f32, kind="ExternalInput")  # *CLS_W
    outd = nc.dram_tensor("out", [1, 2], f32, kind="ExternalOutput")

    with TileContext(nc) as tc:
        with (
            tc.tile_pool(name="cop", bufs=1) as cop,      # det consts
            tc.tile_pool(name="dacc", bufs=1) as dacc,    # det long-lived
            tc.tile_pool(name="dscr", bufs=2) as dscr,    # det scratch
            tc.tile_pool(name="cec", bufs=1) as cec,      # ce consts/accums
            tc.tile_pool(name="big", bufs=4) as bigp,     # ce stream tiles
            tc.tile_pool(name="psum", bufs=1, space="PSUM") as psp,
        ):
            out_sb = cec.tile([1, 2], f32)

            # =========== det constants (tiny DMAs, go first) ===========
            pbf_t = cop.tile([1, 4 * NV], f32)
            nc.sync.dma_start(pbf_t[:], pbf[:])
            tb_t = cop.tile([T, 4], f32)
            nc.sync.dma_start(tb_t[:], tbd[:])
            c1hT_t = cop.tile([C, T], f32)
            nc.sync.dma_start(c1hT_t[:], c1hT[:])
            clT_t = cop.tile([C, NV], f32)
            nc.sync.dma_start(clT_t[:], clT[:])
            cl0_t = cop.tile([128, C], f32)
            nc.sync.dma_start(cl0_t[:], cld[0:128, :])
            cl1_t = cop.tile([128, C], f32)
            nc.sync.dma_start(cl1_t[:], cld[128:256, :])
            iota_t = cop.tile([T, NV], f32)
            nc.sync.dma_start(iota_t[:], iotad[:])
            tbb_t = cop.tile([T, 4 * NV], f32)
            nc.sync.dma_start(tbb_t[:], tbbd[:])
            id128_t = cop.tile([128, 128], f32)
            nc.sync.dma_start(id128_t[:], id128d[:])
            ones32_t = cop.tile([T, T], f32)
            nc.vector.memset(ones32_t[:], 1.0)
            ones128_t = cec.tile([128, 1], f32)
            nc.vector.memset(ones128_t[:], 1.0)

            # ce index/valid consts (label gathers issued after the stream
            # DMAs so their scattered descriptors don't contend with it)
            labidx_t = cec.tile([128, NBLK], i32)
            nc.sync.dma_start(labidx_t[:], labidx[:])
            validm_t = cec.tile([128, NBLK], f32)
            nc.sync.dma_start(validm_t[:], validm[:])

            # =========== det preloop ===========
            # class log-sum-exp over 80 classes (no max-subtract: randn fp32)
            sj = dacc.tile([128, 2], f32)
            for j, cl_t in enumerate((cl0_t, cl1_t)):
                scre = dscr.tile([128, C], f32, tag="scre", name="scre")
                nc.scalar.activation(scre[:], cl_t[:], AF.Exp,
                                     accum_out=sj[:, j:j + 1])
            lse2 = dacc.tile([128, 2], f32)
            nc.scalar.activation(lse2[:], sj[:], AF.Ln)
            # transpose halves -> one [1,256] row, then scale by CLS_W
            lse_row = dacc.tile([1, NV], f32)
            for j in range(2):
                tp_ps = psp.tile([1, 128], f32, tag="tp", name="tp")
                nc.tensor.transpose(tp_ps[:], lse2[:, j:j + 1], id128_t[:])
                nc.vector.tensor_copy(lse_row[0:1, j * 128:(j + 1) * 128], tp_ps[:])
            nc.vector.tensor_scalar_mul(lse_row[:], lse_row[:], CLS_W)

            def bcast32(rhs_ap, n, tag):
                ps = psp.tile([T, n], f32, tag="pbc", name=tag, bufs=2)
                nc.tensor.matmul(ps[:], lhsT=ones32_t[0:1, 0:T], rhs=rhs_ap,
                                 start=True, stop=True)
                return ps

            # pred coords broadcast to [32, 1024] (x1|y1|x2|y2)
            pbb = dacc.tile([T, 4 * NV], f32)
            for h in range(2):
                ps = bcast32(pbf_t[0:1, h * 512:(h + 1) * 512], 512, "pb%d" % h)
                nc.vector.tensor_copy(pbb[:, h * 512:(h + 1) * 512], ps[:])
            px1 = pbb[:, 0 * NV:1 * NV]
            py1 = pbb[:, 1 * NV:2 * NV]
            px2 = pbb[:, 2 * NV:3 * NV]
            py2 = pbb[:, 3 * NV:4 * NV]

            # cls2[t,p] = CLS_W * (lse[p] - cl[p, tc[t]]) ; both already scaled
            lseb_ps = bcast32(lse_row[0:1, :], NV, "lseb")
            clsel_ps = psp.tile([T, NV], f32, tag="clsel", name="clsel")
            nc.tensor.matmul(clsel_ps[:], lhsT=c1hT_t[:], rhs=clT_t[:],
                             start=True, stop=True)
            clsel_sb = dacc.tile([T, NV], f32)
            nc.vector.tensor_copy(clsel_sb[:], clsel_ps[:])
            cls2 = dacc.tile([T, NV], f32)
            nc.vector.tensor_tensor(cls2[:], lseb_ps[:], clsel_sb[:],
                                    op=OP.subtract)

            # target per-partition scalars
            tx1, ty1, tx2, ty2 = (tb_t[:, k:k + 1] for k in range(4))
            tsm = dacc.tile([T, 4], f32)
            nc.vector.tensor_tensor(tsm[:, 0:1], tx2, tx1, op=OP.subtract)
            nc.vector.tensor_tensor(tsm[:, 1:2], ty2, ty1, op=OP.subtract)
            nc.vector.tensor_tensor(tsm[:, 2:3], tsm[:, 0:1], tsm[:, 1:2],
                                    op=OP.mult)
            ta = tsm[:, 2:3]

            def big(tag):
                return dscr.tile([T, NV], f32, tag=tag, name=tag, bufs=1)

            apw = big("apw"); nc.vector.tensor_tensor(apw[:], px2, px1, op=OP.subtract)
            aph = big("aph"); nc.vector.tensor_tensor(aph[:], py2, py1, op=OP.subtract)
            areap = big("areap")
            nc.vector.tensor_tensor(areap[:], apw[:], aph[:], op=OP.mult)
            ltx = big("ltx"); nc.vector.tensor_scalar(ltx[:], px1, tx1, None, op0=OP.max)
            lty = big("lty"); nc.vector.tensor_scalar(lty[:], py1, ty1, None, op0=OP.max)
            rbx = big("rbx"); nc.vector.tensor_scalar(rbx[:], px2, tx2, None, op0=OP.min)
            rby = big("rby"); nc.vector.tensor_scalar(rby[:], py2, ty2, None, op0=OP.min)
            iw = big("iw")
            nc.vector.tensor_tensor(iw[:], rbx[:], ltx[:], op=OP.subtract)
            nc.vector.tensor_scalar(iw[:], iw[:], 0.0, None, op0=OP.max)
            ih = big("ih")
            nc.vector.tensor_tensor(ih[:], rby[:], lty[:], op=OP.subtract)
            nc.vector.tensor_scalar(ih[:], ih[:], 0.0, None, op0=OP.max)
            inter = dacc.tile([T, NV], f32)
            nc.vector.tensor_tensor(inter[:], iw[:], ih[:], op=OP.mult)
            # union = areap + ta - inter  (fused)
            union = dacc.tile([T, NV], f32)
            nc.vector.scalar_tensor_tensor(union[:], areap[:], ta, inter[:],
                                           op0=OP.add, op1=OP.subtract)
            # matching matrix M = inter / max(union, EPS)
            M = dacc.tile([T, NV], f32)
            den = big("den")
            nc.vector.tensor_scalar(den[:], union[:], EPS, None, op0=OP.max)
            nc.vector.reciprocal_approx_fast(den[:], den[:])
            nc.vector.tensor_tensor(M[:], inter[:], den[:], op=OP.mult)
            # giou iou term: inter / (union + EPS)
            ioug = big("ioug")
            nc.vector.tensor_scalar(den[:], union[:], EPS, None, op0=OP.add)
            nc.vector.reciprocal_approx_fast(den[:], den[:])
            nc.vector.tensor_tensor(ioug[:], inter[:], den[:], op=OP.mult)
            # enclosing box term: (areae - union) / (areae + EPS)
            elx = big("elx"); nc.vector.tensor_scalar(elx[:], px1, tx1, None, op0=OP.min)
            ely = big("ely"); nc.vector.tensor_scalar(ely[:], py1, ty1, None, op0=OP.min)
            erx = big("erx"); nc.vector.tensor_scalar(erx[:], px2, tx2, None, op0=OP.max)
            ery = big("ery"); nc.vector.tensor_scalar(ery[:], py2, ty2, None, op0=OP.max)
            ew = big("ew"); nc.vector.tensor_tensor(ew[:], erx[:], elx[:], op=OP.subtract)
            eh = big("eh"); nc.vector.tensor_tensor(eh[:], ery[:], ely[:], op=OP.subtract)
            areae = big("areae"); nc.vector.tensor_tensor(areae[:], ew[:], eh[:], op=OP.mult)
            gt1 = big("gt1"); nc.vector.tensor_tensor(gt1[:], areae[:], union[:], op=OP.subtract)
            nc.vector.tensor_scalar(areae[:], areae[:], EPS, None, op0=OP.add)
            nc.vector.reciprocal_approx_fast(areae[:], areae[:])
            nc.vector.tensor_tensor(gt1[:], gt1[:], areae[:], op=OP.mult)
            # frac - ioug  (giou_loss = 1 + frac - ioug; the +1 is folded into
            # the finalize as GIOU_C per valid match)
            nc.vector.tensor_tensor(gt1[:], gt1[:], ioug[:], op=OP.subtract)

            # smooth L1 (beta=1): huber(d) = 0.5*(ad^2 - relu(ad-1)^2)
            #                              = 0.5*(ad-r)*(ad+r),  r=relu(ad-1)
            # All 4 coords at once on [32,1024] (tbb = targets repeated 256x)
            def wide(tag):
                return dscr.tile([T, 4 * NV], f32, tag=tag, name=tag, bufs=1)

            dw = wide("dw")
            nc.vector.tensor_tensor(dw[:], pbb[:], tbb_t[:], op=OP.subtract)
            ndw = wide("ndw")
            nc.vector.tensor_scalar_mul(ndw[:], dw[:], -1.0)
            adw = wide("adw")
            nc.vector.tensor_tensor(adw[:], dw[:], ndw[:], op=OP.max)
            rw = wide("rw")
            nc.vector.tensor_scalar(rw[:], adw[:], 1.0, 0.0,
                                    op0=OP.subtract, op1=OP.max)
            aprw = wide("aprw")
            nc.vector.tensor_tensor(aprw[:], adw[:], rw[:], op=OP.add)
            amrw = wide("amrw")
            nc.vector.tensor_tensor(amrw[:], adw[:], rw[:], op=OP.subtract)
            qw = wide("qw")
            nc.vector.scalar_tensor_tensor(qw[:], aprw[:], 0.5, amrw[:],
                                           op0=OP.mult, op1=OP.mult)
            sl2 = dscr.tile([T, 2 * NV], f32, tag="sl2", name="sl2", bufs=1)
            nc.vector.tensor_tensor(sl2[:], qw[:, 0:2 * NV], qw[:, 2 * NV:4 * NV],
                                    op=OP.add)
            sl = dacc.tile([T, NV], f32)
            nc.vector.tensor_tensor(sl[:], sl2[:, 0:NV], sl2[:, NV:2 * NV],
                                    op=OP.add)

            # L = GIOU_C*(frac-ioug) + cls2 + COORD_W*L1_W*0.25*sl
            #     (true per-match loss = L + GIOU_C; constant folded into finalize)
            L = dacc.tile([T, NV], f32)
            nc.vector.scalar_tensor_tensor(L[:], gt1[:], GIOU_C, cls2[:],
                                           op0=OP.mult, op1=OP.add)
            nc.vector.scalar_tensor_tensor(L[:], sl[:], COORD_W * L1_W * 0.25,
                                           L[:], op0=OP.mult, op1=OP.add)

            # =========== greedy matching loop ===========
            Sst = dacc.tile([T, 32], f32)
            nc.vector.memset(Sst[:], 0.0)
            ST2 = dacc.tile([T, 32], f32)
            nc.vector.memset(ST2[:], 0.0)
            LN = dacc.tile([T, 2], f32)
            nc.vector.memset(LN[:], 0.0)
            W = dacc.tile([T, 4], f32)
            nc.vector.memset(W[:], 0.0)
            mb = dacc.tile([T, 4], f32)
            sv = dacc.tile([T, 4], f32)
            for it in range(niter):
                # per-row max + row-selected L value and col index
                nc.vector.max(Sst[:, 0:8], M[:])
                E = dscr.tile([T, NV], f32, tag="E", name="E")
                nc.vector.tensor_scalar(E[:], M[:], Sst[:, 0:1], None,
                                        op0=OP.is_equal)
                g1 = dscr.tile([T, NV], f32, tag="g1", name="g1")
                nc.vector.scalar_tensor_tensor(
                    g1[:], E[:], 1.0, L[:], op0=OP.mult, op1=OP.mult,
                    accum_out=Sst[:, 8:9])
                g2 = dscr.tile([T, NV], f32, tag="g2", name="g2")
                nc.vector.scalar_tensor_tensor(
                    g2[:], E[:], 1.0, iota_t[:], op0=OP.mult, op1=OP.mult,
                    accum_out=Sst[:, 9:10])
                # global max gm broadcast to all partitions
                ST = dscr.tile([T, 32], f32, tag="ST", name="ST")
                nc.vector.transpose(ST[:], Sst[:])
                nc.vector.tensor_reduce(W[0:1, 0:1], ST[0:1, :], axis=AX.X,
                                        op=OP.max)
                nc.vector.stream_shuffle(mb[:, 0:1], W[:, 0:1], mask=[0] * 32)
                # sv0 = (rowmax >= max(gm, THRESH) - 1e-6): selected AND valid.
                # Below-thresh iterations skip the row mask; their picks
                # contribute zero, matching the reference exactly.
                nc.vector.tensor_scalar(mb[:, 1:2], mb[:, 0:1], THRESH, -1e-6,
                                        op0=OP.max, op1=OP.add)
                nc.vector.tensor_tensor(sv[:, 0:1], Sst[:, 0:1], mb[:, 1:2],
                                        op=OP.is_ge)
                sv0 = sv[:, 0:1]
                nc.vector.tensor_tensor(LN[:, 1:2], LN[:, 1:2], sv0, op=OP.add)
                nc.vector.tensor_tensor(sv[:, 1:2], sv0, Sst[:, 8:9], op=OP.mult)
                nc.vector.tensor_tensor(LN[:, 0:1], LN[:, 0:1], sv[:, 1:2],
                                        op=OP.add)
                # p* broadcast (DVE transpose+reduce+shuffle)
                nc.vector.tensor_tensor(ST2[:, 0:1], sv0, Sst[:, 9:10],
                                        op=OP.mult)
                ST2T = dscr.tile([T, 32], f32, tag="ST2T", name="ST2T")
                nc.vector.transpose(ST2T[:], ST2[:])
                nc.vector.tensor_reduce(W[0:1, 2:3], ST2T[0:1, :], axis=AX.X,
                                        op=OP.add)
                nc.vector.stream_shuffle(mb[:, 2:3], W[:, 2:3], mask=[0] * 32)
                # mask col p* everywhere and row t* (if valid): M -= (M+1)*oh
                oh = dscr.tile([T, NV], f32, tag="oh", name="oh")
                nc.vector.tensor_scalar(oh[:], iota_t[:], mb[:, 2:3], sv0,
                                        op0=OP.is_equal, op1=OP.add)
                dl = dscr.tile([T, NV], f32, tag="dl", name="dl")
                nc.vector.scalar_tensor_tensor(dl[:], M[:], 1.0, oh[:],
                                               op0=OP.add, op1=OP.mult)
                nc.vector.tensor_tensor(M[:], M[:], dl[:], op=OP.subtract)

            # =========== det finalize ===========
            # det = sum(LN0) + n*(GIOU_C - 2*PEN) + (NV+T)*PEN
            red_ps = psp.tile([T, 2], f32, tag="red", name="red")
            nc.tensor.matmul(red_ps[:], lhsT=ones32_t[:], rhs=LN[:],
                             start=True, stop=True)
            fin = dacc.tile([1, 4], f32)
            nc.vector.tensor_copy(fin[0:1, 0:2], red_ps[0:1, 0:2])
            nc.vector.scalar_tensor_tensor(out_sb[0:1, 1:2], fin[0:1, 1:2],
                                           GIOU_C - 2.0 * PEN, fin[0:1, 0:1],
                                           op0=OP.mult, op1=OP.add)
            nc.vector.tensor_scalar(out_sb[0:1, 1:2], out_sb[0:1, 1:2],
                                    float(PEN * (NV + T)), None, op0=OP.add)

            # =========== LM CE: stream ROWS x 32000 bf16 ===========
            lm3 = lm[:].rearrange("(b p v) -> b p v", p=128, v=V)
            sacc = cec.tile([128, NCHUNKS], f32)
            col = 0
            for b in range(NBLK):
                v0 = 0
                for w in CE_PLAN[b]:
                    ch = bigp.tile([128, w], bf16, tag="ch%d" % w,
                                   name="ch%d" % w, bufs=2)
                    nc.sync.dma_start(ch[:], lm3[b, :, v0:v0 + w])
                    nc.scalar.activation(ch[:], ch[:], AF.Exp,
                                         accum_out=sacc[:, col:col + 1])
                    v0 += w
                    col += 1
            # label-logit gathers (scattered reads; after the stream issues)
            lmflat = lm[:].rearrange("(n o) -> n o", o=1)
            labvh = cec.tile([128, NBLK], bf16)
            for b in range(NBLK):
                nc.gpsimd.indirect_dma_start(
                    out=labvh[:, b:b + 1],
                    out_offset=None,
                    in_=lmflat,
                    in_offset=bass.IndirectOffsetOnAxis(
                        ap=labidx_t[:, b:b + 1], axis=0),
                )
            # lse per row-block: ln(sum of the block's chunk sums)
            n0 = len(CE_PLAN[0])
            s4 = cec.tile([128, NBLK], f32)
            nc.vector.tensor_reduce(s4[:, 0:1], sacc[:, 0:n0], axis=AX.X,
                                    op=OP.add)
            nc.vector.tensor_tensor(s4[:, 1:NBLK], sacc[:, n0:NCHUNKS:2],
                                    sacc[:, n0 + 1:NCHUNKS:2], op=OP.add)
            lse4 = cec.tile([128, NBLK], f32)
            nc.scalar.activation(lse4[:], s4[:], AF.Ln)
            labf = cec.tile([128, NBLK], f32)
            nc.vector.tensor_copy(labf[:], labvh[:])
            ce1 = cec.tile([128, NBLK], f32)
            nc.vector.tensor_tensor(ce1[:], lse4[:], labf[:], op=OP.subtract)
            nc.vector.tensor_tensor(ce1[:], ce1[:], validm_t[:], op=OP.mult)
            rowtot = cec.tile([128, 1], f32)
            nc.vector.tensor_reduce(rowtot[:], ce1[:], axis=AX.X, op=OP.add)
            ce_ps = psp.tile([1, 1], f32, tag="ceps", name="ceps")
            nc.tensor.matmul(ce_ps[:], lhsT=ones128_t[:], rhs=rowtot[:],
                             start=True, stop=True)
            nc.vector.tensor_copy(out_sb[0:1, 0:1], ce_ps[:])

            nc.sync.dma_start(outd[:], out_sb[:])

    nc.finalize()
    return nc


def compute_niter(inputs):
    """Host-side safe iteration bound: simulate the fp32 greedy matching and
    find the last step whose global max is >= THRESH. Steps after that point
    contribute exactly zero to the loss (the max is non-increasing), so
    running max_k + 2 iterations is numerically safe (threshold gaps in the
    data are ~1e-3, far above fp32 rounding differences)."""
    bp = np.asarray(inputs["box_preds"], dtype=np.float32)
    tb = np.asarray(inputs["target_boxes"], dtype=np.float32)
    maxk = 0
    for img in range(B):
        a, bb = bp[img], tb[img]
        area_a = (a[:, 2] - a[:, 0]) * (a[:, 3] - a[:, 1])
        area_b = (bb[:, 2] - bb[:, 0]) * (bb[:, 3] - bb[:, 1])
        lt = np.maximum(a[:, None, :2], bb[None, :, :2])
        rb = np.minimum(a[:, None, 2:], bb[None, :, 2:])
        wh = np.clip(rb - lt, 0, None)
        inter = wh[..., 0] * wh[..., 1]
        union = area_a[:, None] + area_b[None, :] - inter
        M = (inter / np.maximum(union, EPS)).astype(np.float32)
        k = 0
        for i in range(T):
            idx = int(M.argmax())
            m = M.flat[idx]
            p, t = idx // T, idx % T
            if m >= THRESH:
                k = i + 1
            else:
                break
            M[p, :] = -1.0
            M[:, t] = -1.0
        maxk = max(maxk, k)
    return int(min(T, maxk + 1))


def make_in_maps(inputs):
    """Shard full inputs into 8 per-core input maps."""
    import ml_dtypes
    lm_logits = np.asarray(inputs["lm_logits"], dtype=np.float32)
    lm_labels = np.asarray(inputs["lm_labels"])
    class_logits = np.asarray(inputs["class_logits"], dtype=np.float32)
    box_preds = np.asarray(inputs["box_preds"], dtype=np.float32)
    target_labels = np.asarray(inputs["target_labels"])
    target_boxes = np.asarray(inputs["target_boxes"], dtype=np.float32)

    lm2 = lm_logits.reshape(B * S, V).astype(ml_dtypes.bfloat16)
    labs = np.asarray(lm_labels).reshape(B * S).astype(np.int64)

    iota = np.broadcast_to(np.arange(NV, dtype=np.float32), (T, NV)).copy()
    id128 = np.eye(128, dtype=np.float32)

    in_maps = []
    for core in range(NCORES):
        r0 = core * ROWS
        lsl = lm2[r0:r0 + ROWS]
        lb = labs[r0:r0 + ROWS]
        valid = (lb != -100)
        safe = np.where(valid & (lb >= 0) & (lb < V), lb, 0)
        flat = (np.arange(ROWS, dtype=np.int64) * V + safe).astype(np.int32)
        labidx = np.ascontiguousarray(flat.reshape(NBLK, 128).T)        # [128, NBLK]
        validm = np.ascontiguousarray(
            valid.astype(np.float32).reshape(NBLK, 128).T)

        img = core % B
        pb = box_preds[img]                      # [256,4]
        tb = target_boxes[img]                   # [32,4]
        tc = np.clip(target_labels[img].astype(np.int64), 0, C - 1)
        c1hT = np.zeros((C, T), dtype=np.float32)
        c1hT[tc, np.arange(T)] = CLS_W
        cl = class_logits[img]                   # [256,80]

        in_maps.append({
            "lm": np.ascontiguousarray(lsl.reshape(-1)),
            "labidx": labidx,
            "validm": validm,
            "pbf": np.ascontiguousarray(pb.T.reshape(1, 4 * NV)),
            "tb": np.ascontiguousarray(tb),
            "tbb": np.ascontiguousarray(np.repeat(tb, NV, axis=1)),
            "c1hT": c1hT,
            "clT": np.ascontiguousarray(cl.T),
            "cl": np.ascontiguousarray(cl),
            "iota": iota,
            "id128": id128,
        })
    return in_maps


def combine(outs, inputs):
    """All-reduce per-core partial losses on host."""
    lm_labels = np.asarray(inputs["lm_labels"])
    n_valid = max(float((lm_labels.reshape(-1) != -100).sum()), 1.0)
    ce_sum = sum(float(o[0, 0]) for o in outs)
    det_sum = sum(float(outs[c][0, 1]) for c in range(B))
    total = LM_W * (ce_sum / n_valid) + DET_W * det_sum
    return np.array(total, dtype=np.float32)


_NC_CACHE = {}


def kernel(**inputs):
    niter = compute_niter(inputs)
    if niter not in _NC_CACHE:
        _NC_CACHE[niter] = build_nc(niter)
    nc = _NC_CACHE[niter]
    in_maps = make_in_maps(inputs)
    from concourse.bass_utils import run_bass_kernel_spmd
    res = run_bass_kernel_spmd(nc, in_maps, list(range(NCORES)))
    outs = [r["out"] for r in res.results]
    return combine(outs, inputs)
